# revision 66
# baseline (speedup 1.0000x reference)
"""DualQDeformableAttention Trainium2 kernel, v6.

Sharding: 8 cores = 4 batch x 2 query-halves. Per core: build v + bf16
quad table (P1), offsets/weights/indices (P2, overlapped with P1),
dma_gather + weighted reduce (P3), output matmul (P4).

v6 vs v2 baseline (7.9ms -> ~5.0ms profiled core time):
  - Idx staging DRAM round-trip removed. v2 staged int16 idxs to DRAM
    with a scattered 2-byte-element DMA (~524K descriptors, ~2.6ms of
    per-engine DMA busy) and loaded them back. v6 computes the
    16-partition-wrapped, 8x-replicated idx layout directly in SBUF via
    8 selection matmuls on the PE (lhsT = selw[:, j, :], a 0/1 matrix
    with sel[p, j, c] = [p == 16j + c%16]), PSUM -> int16 interleave
    copy into the final gather layout.
  - qtab/gather/weights in bf16 (wg accumulation was already bf16, so
    measured error is unchanged at ~4.2e-3): halves qtab write bytes,
    gather HBM bytes (256B descriptors), and quad-fill DVE write bytes.
  - P1 shifted-row path uses the sub-diagonal si matmul for every tile
    (x=127 quad slots are never gathered, so the zero leak-in is
    harmless), so x3f loads in small chunks on alternating HWDGE rings.
  - quad-fill slot copies split between vector and scalar engines; the
    GpSimd engine is kept free for gathers (its stream is in-order, and
    Pool elementwise ops are slow — putting P2 work there regressed).

Known floors (measured via NTFF profile, core 0):
  - GpSimd DMAGatherAnt desc-gen: ~8.5ns/idx, serial on the Q7 cluster;
    512 calls x 1024 idxs ~= 4.4ms = 88% of the span. Multi-queue SWDGE
    would parallelize Q7 pairs but corrupts data (shared RD-engine race
    in the ucode). num_idxs=2048 crashes the device; keep 1024.
  - Pre-gather prefix ~0.43ms: the full qtab must land before the first
    gather (samples reach anywhere in the image), so P1's v-compute/
    write chain is a hard prefix.
"""

import numpy as np

HEADS = 8
NPOINTS = 4
C = 256
HH = 128
WW = 128
N = HH * WW
NQ = N // 2
D = C // HEADS
NT = NQ // 128          # 64
SLAB = 8                # n-tiles per weight-compute slab
CHUNK_NB = 8
NCHUNK = NT // CHUNK_NB  # 8
GPC = CHUNK_NB * NPOINTS
IDXPC = GPC * 128        # 4096
SROW = NQ * NPOINTS // 16  # 2048 idx cols per (br,h)
QROW2 = HEADS * NPOINTS * D  # 1024 elems (4 KB) per cell row

_PROGRAM = None
LAST_RESULT = None


def _build_program(stage=4):
    import os
    k3max = int(os.environ.get('K3CALLS', '999'))
    nidx_call = int(os.environ.get('NIDX', '1024'))
    nqueues = int(os.environ.get('NQUEUES', '1'))
    _k3 = [0]
    import concourse.bass as bass
    import concourse.mybir as mybir
    from concourse import bacc
    from concourse.tile import TileContext
    from concourse.masks import make_identity

    dt = mybir.dt
    Alu = mybir.AluOpType
    AF = mybir.ActivationFunctionType
    AP = bass.AP
    X = mybir.AxisListType.X

    nc = bacc.Bacc('TRN2', num_swdge_queues=nqueues)

    x1h = nc.dram_tensor('x1h', [C, NQ], dt.float32, kind='ExternalInput')
    x2h = nc.dram_tensor('x2h', [C, NQ], dt.float32, kind='ExternalInput')
    x3f = nc.dram_tensor('x3f', [C, N], dt.float32, kind='ExternalInput')
    wv_d = nc.dram_tensor('wv', [C, C], dt.float32, kind='ExternalInput')
    wcat_d = nc.dram_tensor('wcat', [C, 192], dt.float32, kind='ExternalInput')
    bcat_d = nc.dram_tensor('bcat', [192], dt.float32, kind='ExternalInput')
    wout_d = nc.dram_tensor('wout', [2 * C, C], dt.bfloat16, kind='ExternalInput')
    bout_d = nc.dram_tensor('bout', [C], dt.float32, kind='ExternalInput')
    nx_d = nc.dram_tensor('nx', [128], dt.float32, kind='ExternalInput')
    ny_d = nc.dram_tensor('ny', [NT], dt.float32, kind='ExternalInput')
    si_d = nc.dram_tensor('si', [128, 128], dt.float32, kind='ExternalInput')
    selw_d = nc.dram_tensor('selw', [128, 8 * 128], dt.float32, kind='ExternalInput')
    out_d = nc.dram_tensor('out', [C, NQ], dt.float32, kind='ExternalOutput')

    W16_BR = HEADS * NT * NPOINTS * 4
    W16_H = NT * NPOINTS * 4

    XT = 8                  # t-tiles per x3f load chunk
    QB = 4                  # t-tiles per qtab DMA batch
    SLAB = 4                # n-tiles per P2 slab (overrides module const)

    with TileContext(nc) as tc:
        with tc.tile_pool(name='dram', bufs=1, space='DRAM') as dpool, \
             tc.tile_pool(name='consts', bufs=1) as cpool:

            qtab = dpool.tile([N, QROW2], dt.bfloat16)

            # --- constants ---
            wv_sb = cpool.tile([128, 2, C], dt.float32)
            nc.sync.dma_start(wv_sb[:], wv_d[:].rearrange("(a p) n -> p a n", p=128))
            wcat_sb = cpool.tile([128, 2, 192], dt.float32)
            nc.sync.dma_start(wcat_sb[:], wcat_d[:].rearrange("(a p) n -> p a n", p=128))
            bcat_sb = cpool.tile([128, 192], dt.float32)
            nc.sync.dma_start(bcat_sb[:], AP(tensor=bcat_d, offset=0, ap=[[0, 128], [1, 192]]))
            wout_sb = cpool.tile([128, 4, C], dt.bfloat16)
            nc.sync.dma_start(wout_sb[:], wout_d[:].rearrange("(a p) n -> p a n", p=128))
            bout_sb = cpool.tile([128, 2], dt.float32)
            nc.sync.dma_start(bout_sb[:], bout_d[:].rearrange("(a p) -> p a", p=128))
            nx_sb = cpool.tile([128, 1], dt.float32)
            nc.sync.dma_start(nx_sb[:], nx_d[:].rearrange("(a p) -> p a", p=128))
            ny_sb = cpool.tile([128, NT], dt.float32)
            nc.sync.dma_start(ny_sb[:], AP(tensor=ny_d, offset=0, ap=[[0, 128], [1, NT]]))
            ident = cpool.tile([128, 128], dt.float32)
            make_identity(nc, ident[:])
            si_sb = cpool.tile([128, 128], dt.float32)
            nc.sync.dma_start(si_sb[:], si_d[:])
            selw_sb = cpool.tile([128, 8, 128], dt.float32)
            nc.sync.dma_start(selw_sb[:], selw_d[:])

            # persistent idx + weight stores (filled by P2, read by P3)
            w16 = cpool.tile([128, 2, HEADS, NT, NPOINTS, 4], dt.bfloat16)
            idxth = cpool.tile([128, 2 * HEADS, SROW], dt.int16)

            # ---------------- P1: v + quad table ----------------
            # quad row for cell (y,x): [h, (yy,xx), d] ; slot (yy,xx) holds
            # v[(y+yy)*128 + x+xx]. Rows with x=127 or y=127 are never
            # gathered (bases are clamped to <=126), so their stale/zero
            # slots are harmless — this also makes the si-shift exact
            # enough at every tile boundary.
            with tc.tile_pool(name='p1x', bufs=2) as p1x, \
                 tc.tile_pool(name='p1ps', bufs=2, space='PSUM') as p1ps, \
                 tc.tile_pool(name='p1v', bufs=4) as p1v, \
                 tc.tile_pool(name='p1q', bufs=2) as p1q, \
                 tc.tile_pool(name='p2x', bufs=2) as p2x, \
                 tc.tile_pool(name='p2ps', bufs=2, space='PSUM') as p2ps, \
                 tc.tile_pool(name='p2fp', bufs=2, space='PSUM') as p2fp, \
                 tc.tile_pool(name='p2s', bufs=2) as p2s, \
                 tc.tile_pool(name='p2t', bufs=1) as p2t:
                vts = {}
                qb = None

                # spread the 4 slot copies across vector/scalar/gpsimd so the
                # pre-gather span isn't DVE-bound
                def quad_fill(t, v0, v1):
                    b = t % QB
                    for slot, src in ((0, v0[0]), (1, v0[1]), (2, v1[0]), (3, v1[1])):
                        dst = AP(tensor=qb.tensor,
                                 offset=qb.offset + b * QROW2 + slot * D,
                                 ap=[[qb.ap[0][0], 128], [NPOINTS * D, HEADS], [1, D]])
                        src_ap = src[:].rearrange("p (h d) -> p h d", d=D)
                        if slot == 3:
                            nc.scalar.activation(dst, src_ap, AF.Copy)
                        else:
                            nc.vector.tensor_copy(dst, src_ap)

                for t in range(129):
                    if t < 128:
                        if t % XT == 0:
                            xh = p1x.tile([128, 2, XT * 128], dt.float32, tag='xh', name='xh')
                            cs = slice(t * 128, (t + XT) * 128)
                            ldeng = nc.sync if (t // XT) % 2 == 0 else nc.scalar
                            ldeng.dma_start(xh[:, 0], x3f[0:128, cs])
                            ldeng.dma_start(xh[:, 1], x3f[128:256, cs])
                        tl = (t % XT) * 128
                        vp = p1ps.tile([128, C], dt.float32, tag='vp', name='vp')
                        nc.tensor.matmul(vp[:], xh[:, 0, tl:tl + 128], wv_sb[:, 0], start=True, stop=False)
                        nc.tensor.matmul(vp[:], xh[:, 1, tl:tl + 128], wv_sb[:, 1], start=False, stop=True)
                        vt = p1v.tile([128, C], dt.float32, tag='vt', name='vt')
                        nc.scalar.activation(vt[:], vp[:], AF.Copy)
                        # vtsh[p] = v[cell t*128+p+1]; row 127 leaks zero,
                        # which only lands in never-gathered x=127 slots.
                        vpsh = p1ps.tile([128, C], dt.float32, tag='vpsh', name='vpsh')
                        nc.tensor.matmul(vpsh[:], si_sb[:], vt[:], start=True, stop=True)
                        vtsh = p1v.tile([128, C], dt.float32, tag='vtsh', name='vtsh')
                        nc.scalar.activation(vtsh[:], vpsh[:], AF.Copy)
                        vts[t] = (vt, vtsh)
                    if t == 0:
                        qb = p1q.tile([128, QB, QROW2], dt.bfloat16, tag='qb', name='qb')
                        continue
                    tf = t - 1  # row being filled
                    quad_fill(tf, vts[tf], vts[min(t, 127)])
                    vts.pop(tf - 1, None)
                    if tf % QB == QB - 1 or tf == 127:
                        nb = tf % QB + 1
                        row0 = (tf - nb + 1)
                        dst = AP(tensor=qtab.tensor,
                                 offset=qtab.offset + row0 * 128 * QROW2,
                                 ap=[[QROW2, 128], [128 * QROW2, nb], [1, QROW2]])
                        eng = nc.sync if (tf // QB) % 2 == 0 else nc.scalar
                        eng.dma_start(dst, qb[:, 0:nb, :])
                        if tf != 127:
                            qb = p1q.tile([128, QB, QROW2], dt.bfloat16, tag='qb', name='qb')

                # ---------------- P2: offsets / weights / indices ----------------
                for sl in range(NT // SLAB if stage >= 2 else 0):
                    cs = slice(sl * SLAB * 128, (sl + 1) * SLAB * 128)
                    x1s = p2x.tile([128, 2, SLAB * 128], dt.float32, tag='x1s', name='x1s')
                    nc.sync.dma_start(x1s[:, 0], x1h[0:128, cs])
                    nc.sync.dma_start(x1s[:, 1], x1h[128:256, cs])
                    x2s = p2x.tile([128, 2, SLAB * 128], dt.float32, tag='x2s', name='x2s')
                    nc.scalar.dma_start(x2s[:, 0], x2h[0:128, cs])
                    nc.scalar.dma_start(x2s[:, 1], x2h[128:256, cs])
                    oslab = p2s.tile([128, SLAB, 192], dt.float32, name='oslab')
                    for j in range(SLAB):
                        tl0 = j * 128
                        ops = p2ps.tile([128, 192], dt.float32, name='ops')
                        nc.tensor.matmul(ops[:, 0:96], x1s[:, 0, tl0:tl0 + 128], wcat_sb[:, 0, 0:96], start=True, stop=False)
                        nc.tensor.matmul(ops[:, 0:96], x1s[:, 1, tl0:tl0 + 128], wcat_sb[:, 1, 0:96], start=False, stop=True)
                        nc.tensor.matmul(ops[:, 96:192], x2s[:, 0, tl0:tl0 + 128], wcat_sb[:, 0, 96:192], start=True, stop=False)
                        nc.tensor.matmul(ops[:, 96:192], x2s[:, 1, tl0:tl0 + 128], wcat_sb[:, 1, 96:192], start=False, stop=True)
                        nc.vector.tensor_tensor(out=oslab[:, j], in0=ops[:], in1=bcat_sb[:], op=Alu.add)

                    for br in range(2):
                        base = br * 96

                        def tl(tag, shape=None, dtp=dt.float32):
                            return p2t.tile(shape or [128, SLAB, 32], dtp, tag=tag, name=tag)

                        esl = tl('esl')
                        aw_in = AP(tensor=oslab.tensor, offset=oslab.offset + base + 64,
                                   ap=[oslab.ap[0], [192, SLAB], [1, 32]])
                        nc.scalar.activation(esl[:], aw_in, AF.Exp)
                        ssum = tl('ssum', [128, SLAB])
                        nc.vector.tensor_reduce(op=Alu.add, out=ssum[:], in_=esl[:], axis=X)
                        sinv = tl('sinv', [128, SLAB])
                        nc.vector.reciprocal(sinv[:], ssum[:])
                        es = tl('es')
                        nc.vector.tensor_tensor(
                            out=es[:], in0=esl[:],
                            in1=AP(tensor=sinv.tensor, offset=sinv.offset,
                                   ap=[sinv.ap[0], [1, SLAB], [0, 32]]),
                            op=Alu.mult)

                        offx = AP(tensor=oslab.tensor, offset=oslab.offset + base,
                                  ap=[oslab.ap[0], [192, SLAB], [2, 32]])
                        offy = AP(tensor=oslab.tensor, offset=oslab.offset + base + 1,
                                  ap=[oslab.ap[0], [192, SLAB], [2, 32]])
                        ix = tl('ix')
                        nc.vector.scalar_tensor_tensor(
                            out=ix[:], in0=offx, scalar=128.0,
                            in1=AP(tensor=nx_sb.tensor, offset=nx_sb.offset,
                                   ap=[nx_sb.ap[0], [0, SLAB], [0, 32]]),
                            op0=Alu.mult, op1=Alu.add)
                        iy = tl('iy')
                        nc.vector.scalar_tensor_tensor(
                            out=iy[:], in0=offy, scalar=128.0,
                            in1=AP(tensor=ny_sb.tensor, offset=ny_sb.offset + sl * SLAB,
                                   ap=[ny_sb.ap[0], [1, SLAB], [0, 32]]),
                            op0=Alu.mult, op1=Alu.add)

                        def floorfrac(coord, pfx):
                            half = tl(pfx + 'h')
                            nc.scalar.activation(half[:], coord[:], AF.Copy, bias=-0.5)
                            ci = tl(pfx + 'i', dtp=dt.int32)
                            nc.vector.tensor_copy(ci[:], half[:])
                            cf = tl(pfx + 'f')
                            nc.vector.tensor_copy(cf[:], ci[:])
                            fr = tl(pfx + 'r')
                            nc.vector.tensor_tensor(out=fr[:], in0=coord[:], in1=cf[:], op=Alu.subtract)
                            return cf, fr

                        x0f, fx = floorfrac(ix, 'fx')
                        y0f, fy = floorfrac(iy, 'fy')

                        def slotw(c0f, fr, pfx, eng):
                            ge = tl(pfx + 'ge')
                            eng.tensor_scalar(out=ge[:], in0=c0f[:], scalar1=0.0, scalar2=None, op0=Alu.is_ge)
                            ax = tl(pfx + 'ax')
                            eng.scalar_tensor_tensor(out=ax[:], in0=c0f[:], scalar=126.0,
                                                     in1=ge[:], op0=Alu.is_le, op1=Alu.mult)
                            omf = tl(pfx + 'omf')
                            eng.tensor_scalar(out=omf[:], in0=fr[:], scalar1=-1.0, scalar2=1.0, op0=Alu.mult, op1=Alu.add)
                            s0 = tl(pfx + 's0')
                            eng.tensor_tensor(out=s0[:], in0=omf[:], in1=ax[:], op=Alu.mult)
                            t0 = tl(pfx + 't0')
                            eng.scalar_tensor_tensor(out=t0[:], in0=c0f[:], scalar=-1.0,
                                                     in1=fr[:], op0=Alu.is_equal, op1=Alu.mult)
                            eng.tensor_tensor(out=s0[:], in0=s0[:], in1=t0[:], op=Alu.add)
                            s1 = tl(pfx + 's1')
                            eng.tensor_tensor(out=s1[:], in0=fr[:], in1=ax[:], op=Alu.mult)
                            t1 = tl(pfx + 't1')
                            eng.scalar_tensor_tensor(out=t1[:], in0=c0f[:], scalar=127.0,
                                                     in1=omf[:], op0=Alu.is_equal, op1=Alu.mult)
                            eng.tensor_tensor(out=s1[:], in0=s1[:], in1=t1[:], op=Alu.add)
                            return s0, s1

                        sx0, sx1 = slotw(x0f, fx, 'sx', nc.vector)
                        sy0, sy1 = slotw(y0f, fy, 'sy', nc.vector)

                        ay0 = tl('ay0')
                        nc.vector.tensor_tensor(out=ay0[:], in0=sy0[:], in1=es[:], op=Alu.mult)
                        ay1 = tl('ay1')
                        nc.vector.tensor_tensor(out=ay1[:], in0=sy1[:], in1=es[:], op=Alu.mult)

                        for (qi, ayv, sxv) in ((0, ay0, sx0), (1, ay0, sx1),
                                               (2, ay1, sx0), (3, ay1, sx1)):
                            wdst = AP(tensor=w16.tensor,
                                      offset=w16.offset + br * W16_BR + sl * SLAB * (NPOINTS * 4) + qi,
                                      ap=[w16.ap[0], [NPOINTS * 4, SLAB], [W16_H, HEADS], [4, NPOINTS]])
                            win0 = AP(tensor=ayv.tensor, offset=ayv.offset,
                                      ap=[ayv.ap[0], [32, SLAB], [NPOINTS, HEADS], [1, NPOINTS]])
                            win1 = AP(tensor=sxv.tensor, offset=sxv.offset,
                                      ap=[sxv.ap[0], [32, SLAB], [NPOINTS, HEADS], [1, NPOINTS]])
                            nc.vector.tensor_tensor(out=wdst, in0=win0, in1=win1, op=Alu.mult)

                        xb = tl('xb')
                        nc.vector.tensor_scalar(out=xb[:], in0=x0f[:], scalar1=0.0, scalar2=126.0, op0=Alu.max, op1=Alu.min)
                        yb = tl('yb')
                        nc.vector.tensor_scalar(out=yb[:], in0=y0f[:], scalar1=0.0, scalar2=126.0, op0=Alu.max, op1=Alu.min)
                        idxf = tl('idxf')
                        nc.vector.scalar_tensor_tensor(out=idxf[:], in0=yb[:], scalar=128.0, in1=xb[:], op0=Alu.mult, op1=Alu.add)

                        # fold+replicate into the 16-wrapped gather layout:
                        # idxth[16jj+pp, brh, 32t+8p+j] = idxf[16j+pp, t, h, p]
                        # via out[P, f] = sum_p selw[p, j, P] * idxf[p, f].
                        idxf_flat = AP(tensor=idxf.tensor, offset=idxf.offset,
                                       ap=[idxf.ap[0], [1, SLAB * 32]])
                        for j in range(8):
                            fp2 = p2fp.tile([128, SLAB * 32], dt.float32, tag='fp2', name='fp2')
                            nc.tensor.matmul(fp2[:], selw_sb[:, j, :], idxf_flat, start=True, stop=True)
                            idst = AP(tensor=idxth.tensor,
                                      offset=idxth.offset + br * (HEADS * SROW)
                                      + sl * SLAB * NPOINTS * 8 + j,
                                      ap=[idxth.ap[0], [SROW, HEADS], [NPOINTS * 8, SLAB], [8, NPOINTS]])
                            isrc = AP(tensor=fp2.tensor, offset=fp2.offset,
                                      ap=[fp2.ap[0], [NPOINTS, HEADS], [32, SLAB], [1, NPOINTS]])
                            # scalar engine: f32 PSUM -> int16 write-cast keeps
                            # these 256 copies off the DVE stream (values are
                            # exact integers, so the cast mode is immaterial)
                            nc.scalar.activation(idst, isrc, AF.Copy)

            # ---------------- P3 + P4 ----------------
            with tc.tile_pool(name='p3g', bufs=5) as p3g, \
                 tc.tile_pool(name='p3w', bufs=2) as p3w, \
                 tc.tile_pool(name='p3o', bufs=2) as p3o, \
                 tc.tile_pool(name='p4ps', bufs=4, space='PSUM') as p4ps, \
                 tc.tile_pool(name='p4t', bufs=1) as p4t, \
                 tc.tile_pool(name='p4f', bufs=2, space='PSUM') as p4f, \
                 tc.tile_pool(name='p4o', bufs=2) as p4o:
                nsub = IDXPC // nidx_call
                for ch in range(NCHUNK if stage >= 3 else 0):
                    outcat = p3o.tile([128, CHUNK_NB, 512], dt.float32, name='outcat')
                    for br in range(2):
                        for h in range(HEADS):
                            if _k3[0] >= k3max:
                                continue
                            _k3[0] += 1
                            brh = br * HEADS + h
                            gat = p3g.tile([128, GPC, 128], dt.bfloat16, tag='gat', name='gat')
                            hsrc = AP(tensor=qtab.tensor, offset=qtab.offset + h * 128,
                                      ap=[[QROW2, N], [1, 128]])
                            ib0 = ch * (IDXPC // 16)
                            for sub in range(nsub):
                                g0 = sub * (GPC // nsub)
                                i0 = ib0 + sub * (nidx_call // 16)
                                nc.gpsimd.dma_gather(
                                    out_ap=gat[:, g0:g0 + GPC // nsub, :],
                                    in_ap=hsrc,
                                    idxs_ap=idxth[:, brh, i0:i0 + nidx_call // 16],
                                    num_idxs=nidx_call, num_idxs_reg=nidx_call,
                                    elem_size=128, elem_step=QROW2,
                                    queue_num=(brh * nsub + sub) % nqueues)
                            wg = p3w.tile([128, GPC, 128], dt.bfloat16, tag='wg', name='wg')
                            win = AP(tensor=w16.tensor,
                                     offset=w16.offset + br * W16_BR + h * W16_H
                                     + ch * CHUNK_NB * NPOINTS * 4,
                                     ap=[w16.ap[0], [4, GPC], [1, 4], [0, D]])
                            nc.vector.tensor_tensor(
                                out=wg[:].rearrange("p g (q d) -> p g q d", d=D),
                                in0=gat[:].rearrange("p g (q d) -> p g q d", d=D),
                                in1=win, op=Alu.mult)
                            rin = AP(tensor=wg.tensor, offset=wg.offset,
                                     ap=[wg.ap[0], [NPOINTS * 128, CHUNK_NB], [1, D], [D, 16]])
                            rout = AP(tensor=outcat.tensor,
                                      offset=outcat.offset + br * 256 + h * D,
                                      ap=[outcat.ap[0], [512, CHUNK_NB], [1, D]])
                            nc.vector.tensor_reduce(op=Alu.add, out=rout, in_=rin, axis=X)

                    if stage < 4:
                        continue
                    ocT = p4t.tile([128, 4, CHUNK_NB * 128], dt.bfloat16, name='ocT')
                    for nb in range(CHUNK_NB):
                        for k in range(4):
                            tp = p4ps.tile([128, 128], dt.float32, tag='tp', name='tp')
                            nc.tensor.transpose(tp[:], outcat[:, nb, k * 128:(k + 1) * 128], ident[:])
                            nc.scalar.activation(ocT[:, k, nb * 128:(nb + 1) * 128], tp[:], AF.Copy)
                    for chl in range(2):
                        for wnd in range(CHUNK_NB * 128 // 512):
                            fp = p4f.tile([128, 512], dt.float32, tag='fp', name='fp')
                            for k in range(4):
                                nc.tensor.matmul(
                                    fp[:], wout_sb[:, k, chl * 128:(chl + 1) * 128],
                                    ocT[:, k, wnd * 512:(wnd + 1) * 512],
                                    start=(k == 0), stop=(k == 3))
                            ob = p4o.tile([128, 512], dt.float32, tag='ob', name='ob')
                            nc.scalar.activation(ob[:], fp[:], AF.Identity, bias=bout_sb[:, chl:chl + 1])
                            col0 = ch * (CHUNK_NB * 128) + wnd * 512
                            nc.sync.dma_start(out_d[chl * 128:(chl + 1) * 128, col0:col0 + 512], ob[:])

    nc.compile()
    return nc


def _get_program():
    global _PROGRAM
    if _PROGRAM is None:
        import os
        _PROGRAM = _build_program(stage=int(os.environ.get('KSTAGE', '4')))
    return _PROGRAM


def make_in_maps(x1, x2, x3, inputs):
    wcat = np.ascontiguousarray(np.concatenate(
        [np.asarray(inputs['Woff1']), np.asarray(inputs['Waw1']),
         np.asarray(inputs['Woff2']), np.asarray(inputs['Waw2'])],
        axis=1).astype(np.float32))
    bcat = np.ascontiguousarray(np.concatenate(
        [np.asarray(inputs['boff1']), np.asarray(inputs['baw1']),
         np.asarray(inputs['boff2']), np.asarray(inputs['baw2'])]
    ).astype(np.float32))
    wv = np.ascontiguousarray(np.asarray(inputs['Wv'], dtype=np.float32))
    wout_f32 = np.asarray(inputs['Wout'], dtype=np.float32)
    try:
        import ml_dtypes
        wout = np.ascontiguousarray(wout_f32.astype(ml_dtypes.bfloat16))
    except ImportError:
        # round-to-nearest-even f32 -> bf16, kept as a uint16 view
        u = wout_f32.view(np.uint32)
        rounded = ((u + 0x7FFF + ((u >> 16) & 1)) >> 16).astype(np.uint16)
        wout = np.ascontiguousarray(rounded)
    boutv = np.ascontiguousarray(np.asarray(inputs['bout'], dtype=np.float32))
    nx = np.arange(128, dtype=np.float32)
    p = np.arange(128)[:, None]
    jj, cc = np.divmod(np.arange(8 * 128), 128)
    selw = (p == 16 * jj[None, :] + (cc[None, :] % 16)).astype(np.float32)
    selw = np.ascontiguousarray(selw)

    in_maps = []
    for core in range(8):
        b, half = core // 2, core % 2
        x1f = x1[b].reshape(C, N)
        x2f = x2[b].reshape(C, N)
        in_maps.append({
            'x1h': np.ascontiguousarray(x1f[:, half * NQ:(half + 1) * NQ]),
            'x2h': np.ascontiguousarray(x2f[:, half * NQ:(half + 1) * NQ]),
            'x3f': np.ascontiguousarray(x3[b].reshape(C, N)),
            'wv': wv, 'wcat': wcat, 'bcat': bcat,
            'wout': wout, 'bout': boutv,
            'nx': nx,
            'ny': (half * NT + np.arange(NT)).astype(np.float32),
            'si': np.eye(128, 128, -1, dtype=np.float32),
            'selw': selw,
        })
    return in_maps


def assemble_output(results, B, ncores):
    out = np.zeros((B, C, HH, WW), dtype=np.float32)
    for core in range(ncores):
        b, half = core // 2, core % 2
        out[b].reshape(C, N)[:, half * NQ:(half + 1) * NQ] = results[core]['out']
    return out


def kernel(x1, x2, x3, Wv, Woff1, boff1, Woff2, boff2, Waw1, baw1, Waw2, baw2, Wout, bout):
    from concourse.bass_utils import run_bass_kernel_spmd

    x1 = np.asarray(x1, dtype=np.float32)
    x2 = np.asarray(x2, dtype=np.float32)
    x3 = np.asarray(x3, dtype=np.float32)
    B = x1.shape[0]
    nc = _get_program()
    in_maps = make_in_maps(x1, x2, x3, {
        'Woff1': Woff1, 'boff1': boff1, 'Woff2': Woff2, 'boff2': boff2,
        'Waw1': Waw1, 'baw1': baw1, 'Waw2': Waw2, 'baw2': baw2,
        'Wv': Wv, 'Wout': Wout, 'bout': bout})
    import os as _os
    ncores = int(_os.environ.get('NCORES', '8'))
    res = run_bass_kernel_spmd(nc, in_maps[:ncores], core_ids=list(range(ncores)))
    global LAST_RESULT
    LAST_RESULT = res
    return assemble_output([res.results[c] for c in range(ncores)], B, ncores)
